# revision 36
# baseline (speedup 1.0000x reference)
"""Trainium2 Bass kernel for an attention layer.

Computes, per batch element b (8 batches, one per NeuronCore):
    q = Wq @ x[b]            # [256, 2048]
    k = Wk @ x[b]            # [256, 2048]
    v = Wv @ x[b]            # [512, 2048]
    sim = k.T @ q            # [2048, 2048]
    attn = softmax(sim, -1)
    out[b] = (v @ attn).T    # [2048, 512]

Sharding: data-parallel over batch B=8 across the 8 cores; no collectives.

Per-core dataflow (fp16 matmuls, fp32 PSUM accumulation) with two headline
optimizations over the plain fp16 pipeline:

1. h-split emission order for DMA slack.  The first NSPLIT key-tiles'
   sim work is split by j-halves: proj jc0/jc1 -> sim h0 (cols 0..1023)
   for tiles 0..7 -> proj jc2/jc3 -> sim h1 + stats + vT for tiles 0..7
   -> tiles 8..15 whole.  The 32 h0 matmuls consume no fresh DMA input,
   giving the x jc1/jc2/jc3 and wv transfers a wide window; pieces are
   queued per-queue in consumption order (3 queues: sync/scalar HWDGE,
   gpsimd SWDGE), so the PE's input-DMA stalls drop from ~3.8us to
   ~1-2us of first-piece jitter.

2. fp8 DoubleRow for half the attention*V contraction.  A DoubleRow
   e4m3 matmul contracts 256 keys per 512-cycle instruction (2 fp8
   weights/cell; measured at bf16's per-instruction cost, i.e. true 2x
   MACs), so key-tiles stored fp8 cost half the out-phase matmul
   slots.  Accuracy only allows a partial conversion: e4m3's ~2.6% rms
   quantization noise on the V operand adds ~2.5e-2 L2 error if
   applied to the whole contraction (gate: 2e-2), so only the LAST
   2*N8PAIR key-tiles go fp8: rel err ~= sqrt(3.3e-3^2 +
   (2.5e-2)^2 * N8PAIR/8); N8PAIR=4 measures 1.863e-2 on HW and saves
   64 of 256 out-phase matmul slots.  The attention-weight side is
   nearly free: p8 = e4m3(exp_s * 128/emax) puts each row's max weight
   exactly on 128 (an e4m3 grid point, zero quantization error on the
   dominant weight), and v8 = vp * rden * emax/128 so the emax factor
   cancels exactly in the p8 @ v8 product.  v8 = 32 v / denrow stays
   within +-166 (TRN e4m3 maps >248 to Inf).  Wv is host-scaled by
   4096 so fp8 and bf16 tile contributions share one PSUM scale; the
   final copies multiply by 2^-12.

Engine balance (the PE must never wait):
  - All softmax denominators ride the ACT accumulator (both halves);
    the DVE never reduces exp_s for sums.
  - The fp8 tiles are the LAST 8: their conversion chains (per-half
    emax reduces + p8/v8 scalar-muls, ~5us of DVE per tile vs 2.6us of
    PE) drain into the out phase, whose first groups lead with the
    early-ready bf16 tiles; the final output copies run on ACT
    (activation Copy with immediate scale), which idles there, keeping
    the DVE free for the conversion backlog.
  - sim PSUM pool has 3 bufs; the vT accumulators share the 2-buf
    "acc" pool (8 banks total).  After the sim pool closes, the out
    phase runs 8 accumulation groups deep (2 on "acc" + 6 on the
    out_psum2 pool reusing the sim banks), enough runway to hide the
    fp8 conversion backlog and the sim->out release barrier (-4us).

Timing notes (8-core SPMD; the chip alternates between 2.4 GHz and a
2.0 GHz chip-wide power-throttle state, so a 512-col MM paces at
216-259 ns and identical builds measure 120-146us):
  - ~7us fixed NEFF/engine-init preamble precedes the first user op;
    3 wide + 7 short warmup matmuls on a zeroed tile bridge engine
    release to the first input pieces landing, and the HAM clock-ramp
    window (~3.4us) overlaps real projection work.
  - 448 real matmul slots (64 proj + 128 sim + 64 vT + 192 out) at
    216 ns = 96.8us of PE floor at 2.4 GHz.
  - The final output tile is halved into ACT+DVE scaled casts whose
    DMAs trigger on two different queues.
"""

import os

import numpy as np

import concourse.tile as tile
from concourse import bacc, mybir
from concourse.bass_utils import run_bass_kernel_spmd

B = 8
C_IN = 512
C_OUT = 512
C_KEY = 256
N = 2048
P = 128

F32 = mybir.dt.float32
F16 = mybir.dt.float16
BF16 = mybir.dt.bfloat16
E4 = mybir.dt.float8e4

DR = mybir.MatmulPerfMode.DoubleRow

NT_CIN = C_IN // P  # 4 tiles over input channels
NT_CK = C_KEY // P  # 2 tiles over key channels
NT_N = N // P  # 16 tiles over sequence positions
JC = 512  # matmul output chunk (one PSUM bank of fp32)
NJC = N // JC  # 4 chunks over the j axis
HC = 1024  # softmax processing chunk (half row block)
NHC = N // HC

EXP_SHIFT = -65.0  # global logit shift; row maxes are ~[38, 103] for this
# problem's N(0,1) inputs, and bf16/fp32 exponent range absorbs e^+-40

# Number of fp8 DoubleRow key-tile pairs (0..4).  Each pair halves 32 of the
# out-phase matmul slots.  Accuracy: ~sqrt(3.3e-3^2 + 6.5e-4 * n8).
N8PAIR = int(os.environ.get("KERNEL_N8PAIR", "4"))
NF8 = 2 * N8PAIR
NSPLIT = 8  # tiles 0..7 are h-split (their h0 sim runs before proj jc2/3)

OUT_SCALE = 1.0 / 4096.0  # undo the host-side Wv*4096 scale


def _build_program(n8pair=None):
    n8 = N8PAIR if n8pair is None else n8pair
    nc = bacc.Bacc("TRN2", target_bir_lowering=False, debug=False)

    # Host-packed inputs: every DRAM tensor is laid out so each SBUF
    # partition's data is one long contiguous run (DMA cost is dominated by
    # descriptor count; short runs halve the effective queue bandwidth).
    #   x:   [128, 4*4*512]  ([p, jc, ct, n] flattened)
    #   wqk: [128, 2*2*4*128]  ([p, w, ckt, ct, m] flattened)
    #   wv:  [128, 4*512]    ([p, ct, co] flattened), host-scaled by 4096
    x_d = nc.dram_tensor(
        "x", [P, NJC * NT_CIN * JC], F16, kind="ExternalInput"
    ).ap()
    wqk_d = nc.dram_tensor(
        "wqk", [P, 2 * NT_CIN * C_KEY], F16, kind="ExternalInput"
    ).ap()
    wv_d = nc.dram_tensor("wv", [P, NT_CIN * C_OUT], F16, kind="ExternalInput").ap()
    out_d = nc.dram_tensor("out", [N, C_OUT], BF16, kind="ExternalOutput").ap()

    with tile.TileContext(nc) as tc:
        _emit_kernel(tc, out_d, x_d, wqk_d, wv_d, n8)

    nc.compile()
    return nc


def _emit_kernel(tc, out_d, x_d, wqk_d, wv_d, n8):
    nc = tc.nc
    Exp = mybir.ActivationFunctionType.Exp
    AxisX = mybir.AxisListType.X
    Add = mybir.AluOpType.add
    Max = mybir.AluOpType.max
    nf8 = 2 * n8

    with (
        tc.tile_pool(name="persist", bufs=1) as persist,
        tc.tile_pool(name="stats", bufs=8) as stats,
        tc.tile_pool(name="ostage", bufs=6) as ostage,
    ):
        # ---- constants & warmup source (vector ops run before its DMAs) ----
        warm_src = persist.tile([P, JC], F16, tag="warm_src")
        nc.vector.memset(warm_src, 0.0)
        shift_bias = persist.tile([P, 1], F32, tag="shift")
        nc.vector.memset(shift_bias, EXP_SHIFT)

        # ---- input staging tiles ----
        x4 = persist.tile([P, NJC, NT_CIN, JC], F16, tag="x4")
        wqk_s = persist.tile([P, 2, NT_CK, NT_CIN, P], F16, tag="wqk")
        wv_s = persist.tile([P, NT_CIN, C_OUT], F16, tag="wv")

        def xp(ct, lo, hi):
            jc, r = divmod(lo, JC)
            assert hi - lo <= JC - r
            return x4[:, jc, ct, r : r + (hi - lo)]

        def x_piece(eng, jc, ct0, nt):
            return eng.dma_start(
                out=x4[:, jc, ct0 : ct0 + nt, :],
                in_=x_d[
                    :,
                    (jc * NT_CIN + ct0) * JC : (jc * NT_CIN + ct0 + nt) * JC,
                ].rearrange("p (t n) -> p t n", t=nt),
            )

        def wqk_piece(eng, w, ckt):
            WB = NT_CIN * P
            base = (w * NT_CK + ckt) * WB
            return eng.dma_start(
                out=wqk_s[:, w, ckt, :, :],
                in_=wqk_d[:, base : base + WB].rearrange("p (t m) -> p t m", t=NT_CIN),
            )

        # DMA schedule: per-queue FIFO order == consumption order (3 HW
        # queues: sync/scalar HWDGE + gpsimd SWDGE).
        # criticals: wqk00+x jc0 by ~9us, wqk01 ~10.5, wqk10 ~11.7,
        # wqk11 ~12.4, x jc1 ~13.5, x jc2 ~23, x jc3 ~26.5, wv ~30.
        x_piece(nc.sync, 0, 0, 1)
        wqk_piece(nc.scalar, 0, 0)
        x_piece(nc.gpsimd, 0, 1, 1)
        x_piece(nc.sync, 0, 2, 1)
        wqk_piece(nc.scalar, 0, 1)
        x_piece(nc.gpsimd, 0, 3, 1)
        wqk_piece(nc.sync, 1, 0)
        x_piece(nc.scalar, 1, 0, 2)
        wqk_piece(nc.gpsimd, 1, 1)
        x_piece(nc.sync, 1, 2, 2)
        x_piece(nc.sync, 2, 0, NT_CIN)
        x_piece(nc.gpsimd, 3, 0, NT_CIN)
        nc.scalar.dma_start(
            out=wv_s, in_=wv_d.rearrange("p (t m) -> p t m", t=NT_CIN)
        )
        # preload ACT's Exp table during the DMA wait (one-time ~1.3us load)
        act_warm = stats.tile([P, 1], F32, tag="actwarm")
        nc.scalar.activation(
            out=act_warm, in_=shift_bias, func=Exp, bias=shift_bias, scale=1.0
        )

        # ---- persistent compute tiles ----
        qs = [
            persist.tile([P, N], F16, tag=f"q{t}", name=f"q{t}") for t in range(NT_CK)
        ]
        ks = [
            persist.tile([P, N], F16, tag=f"k{t}", name=f"k{t}") for t in range(NT_CK)
        ]
        exp_s = [
            persist.tile([P, N], BF16, tag=f"e{it}", name=f"e{it}")
            for it in range(NT_N)
        ]
        vts = [
            persist.tile([P, C_OUT], BF16, tag=f"vt{it}", name=f"vt{it}")
            for it in range(NT_N)
        ]
        # fp8 DoubleRow pair tiles: plane = key-tile parity within the pair.
        # fp8 tiles are the LAST nf8 key-tiles: their conversion chains run
        # while earlier bf16 tiles already feed the out phase.
        f8_base = NT_N - nf8
        exp8 = [
            persist.tile([P, 2, N], E4, tag=f"e8{tp}", name=f"e8{tp}")
            for tp in range(n8)
        ]
        vt8 = [
            persist.tile([P, 2, C_OUT], E4, tag=f"v8{tp}", name=f"v8{tp}")
            for tp in range(n8)
        ]
        # per-tile softmax partial sums (ACT accumulates both halves)
        dpall = persist.tile([P, NT_N, 2], F32, tag="dpall")
        # per-half exp row-maxes for the fp8 tiles (the h0 half runs during
        # the tile's own h1 matmuls, shortening the tail conversion chain)
        em2 = persist.tile([P, max(nf8, 1), 2], F32, tag="em2")

        def sim_half(simp, it, h, name):
            sh = simp.tile([P, HC], F32, tag="sim", name=name)
            for jc in range(HC // JC):
                for ckt in range(NT_CK):
                    nc.tensor.matmul(
                        out=sh[:, jc * JC : (jc + 1) * JC],
                        lhsT=ks[ckt][:, it * P : (it + 1) * P],
                        rhs=qs[ckt][
                            :, (h * HC + jc * JC) : (h * HC + (jc + 1) * JC)
                        ],
                        start=(ckt == 0),
                        stop=(ckt == NT_CK - 1),
                    )
            return sh

        def vt_matmuls(accp, it):
            vp = accp.tile([P, C_OUT], F32, tag="acc", name=f"vp{it}")
            for ct in range(NT_CIN):
                nc.tensor.matmul(
                    out=vp,
                    lhsT=xp(ct, it * P, (it + 1) * P),
                    rhs=wv_s[:, ct, :],
                    start=(ct == 0),
                    stop=(ct == NT_CIN - 1),
                )
            return vp

        # stats+scale chain shared by all tiles; fp8 tiles additionally
        # derive the DoubleRow operands from exp_s/vp:
        #   emax = rowmax(exp_s)      (bf16 SBUF read, no PSUM retention)
        #   t0 = emax/128, rs = 1/t0  (p8 = exp_s*rs puts the row max at 128,
        #                              an exact e4m3 grid point)
        #   rden8 = rden*t0           (so rs cancels exactly in p8 @ v8)
        def tile_stats(accp, it):
            fp8 = it >= f8_base
            den = stats.tile([P, 1], F32, tag="den")
            rden = stats.tile([P, 1], F32, tag="rden")
            nc.vector.tensor_reduce(
                out=den, in_=dpall[:, it, :], axis=AxisX, op=Add
            )
            nc.vector.reciprocal(out=rden, in_=den)
            vp = vt_matmuls(accp, it)
            if fp8:
                fi = it - f8_base
                emax = stats.tile([P, 1], F32, tag="emax")
                nc.vector.tensor_reduce(
                    out=emax, in_=em2[:, fi, :], axis=AxisX, op=Max
                )
                t0 = stats.tile([P, 1], F32, tag="t0")
                nc.vector.tensor_scalar_mul(t0, emax, 1.0 / 128.0)
                rs = stats.tile([P, 1], F32, tag="rs")
                nc.vector.reciprocal(out=rs, in_=t0)
                rden8 = stats.tile([P, 1], F32, tag="rden8")
                nc.vector.tensor_scalar_mul(rden8, rden, t0)
                nc.vector.tensor_scalar_mul(vt8[fi // 2][:, fi % 2, :], vp, rden8)
                nc.vector.tensor_scalar_mul(
                    exp8[fi // 2][:, fi % 2, :], exp_s[it], rs
                )
            else:
                nc.vector.tensor_scalar_mul(vts[it], vp, rden)

        with tc.tile_pool(name="acc_psum", bufs=2, space="PSUM") as accp:
            # PE warmup bridging engine-release to the first input landing;
            # the HAM clock ramp overlaps the first real projections.
            warm_ps = accp.tile([P, JC], F32, tag="acc", name="warm_ps")
            for i in range(10):
                nc.tensor.matmul(
                    out=warm_ps[:, 0 : (JC if i < 3 else JC // 4)],
                    lhsT=warm_src[:, 0:P],
                    rhs=warm_src[:, 0 : (JC if i < 3 else JC // 4)],
                    start=True,
                    stop=True,
                )

            def proj_jc(jc):
                for w, dst in ((0, qs), (1, ks)):
                    for ckt in range(NT_CK):
                        ps = accp.tile([P, JC], F32, tag="acc", name=f"pj{jc}{w}{ckt}")
                        for ct in range(NT_CIN):
                            nc.tensor.matmul(
                                out=ps,
                                lhsT=wqk_s[:, w, ckt, ct, :],
                                rhs=x4[:, jc, ct, :],
                                start=(ct == 0),
                                stop=(ct == NT_CIN - 1),
                            )
                        nc.vector.tensor_copy(
                            out=dst[ckt][:, jc * JC : (jc + 1) * JC], in_=ps
                        )

            proj_jc(0)
            proj_jc(1)

            with tc.tile_pool(name="sim_psum", bufs=3, space="PSUM") as simp:
                # ---- h0 block: sim cols 0..1023 for the split tiles ----
                for it in range(NSPLIT):
                    sh = sim_half(simp, it, 0, f"s0_{it}")
                    nc.scalar.activation(
                        out=exp_s[it][:, 0:HC],
                        in_=sh,
                        func=Exp,
                        bias=shift_bias,
                        scale=1.0,
                        accum_out=dpall[:, it, 0:1],
                    )

                proj_jc(2)
                proj_jc(3)

                # ---- h1 block: sim cols 1024..2047 + stats + vT ----
                for it in range(NSPLIT):
                    sh = sim_half(simp, it, 1, f"s1_{it}")
                    nc.scalar.activation(
                        out=exp_s[it][:, HC:N],
                        in_=sh,
                        func=Exp,
                        bias=shift_bias,
                        scale=1.0,
                        accum_out=dpall[:, it, 1:2],
                    )
                    tile_stats(accp, it)

                # ---- tiles 8..15: whole-row processing; the fp8 tiles'
                # conversion chains drain into the out-phase runway ----
                for it in range(NSPLIT, NT_N):
                    for h in range(NHC):
                        sh = sim_half(simp, it, h, f"sf{it}_{h}")
                        nc.scalar.activation(
                            out=exp_s[it][:, h * HC : (h + 1) * HC],
                            in_=sh,
                            func=Exp,
                            bias=shift_bias,
                            scale=1.0,
                            accum_out=dpall[:, it, h : h + 1],
                        )
                        if it >= f8_base:
                            nc.vector.tensor_reduce(
                                out=em2[:, it - f8_base, h : h + 1],
                                in_=exp_s[it][:, h * HC : (h + 1) * HC],
                                axis=AxisX,
                                op=Max,
                            )
                    tile_stats(accp, it)

            # ---- out[m, co]: bf16 tiles via fp16 MMs, fp8 pairs via
            # DoubleRow (256-key contraction per instruction) ----
            def out_group(pool, tg, mt):
                po = pool.tile([P, C_OUT], F32, tag=tg, name=f"po{mt}")
                nmm = (NT_N - nf8) + n8
                mi = 0
                for it in range(f8_base):
                    nc.tensor.matmul(
                        out=po,
                        lhsT=exp_s[it][:, mt * P : (mt + 1) * P],
                        rhs=vts[it],
                        start=(mi == 0),
                        stop=(mi == nmm - 1),
                    )
                    mi += 1
                for tp in range(n8):
                    nc.tensor.matmul(
                        out=po,
                        lhsT=exp8[tp][:, :, mt * P : (mt + 1) * P],
                        rhs=vt8[tp],
                        start=(mi == 0),
                        stop=(mi == nmm - 1),
                        perf_mode=DR,
                    )
                    mi += 1
                ot = ostage.tile([P, C_OUT], BF16, tag="ostage", name=f"ot{mt}")
                Copy = mybir.ActivationFunctionType.Copy
                if mt < NT_N - 1:
                    # scaled final copy on ACT (idle in the out phase; the
                    # DVE is busy draining the fp8 conversion backlog)
                    deng = nc.sync if mt % 2 == 0 else nc.scalar
                    nc.scalar.activation(
                        out=ot, in_=po, func=Copy, bias=0.0, scale=OUT_SCALE
                    )
                    deng.dma_start(out=out_d[mt * P : (mt + 1) * P, :], in_=ot)
                else:
                    # final tile: halve it, casts split across ACT and DVE,
                    # DMAs on two different queues
                    h = C_OUT // 2
                    nc.scalar.activation(
                        out=ot[:, 0:h],
                        in_=po[:, 0:h],
                        func=Copy,
                        bias=0.0,
                        scale=OUT_SCALE,
                    )
                    nc.sync.dma_start(
                        out=out_d[mt * P : (mt + 1) * P, 0:h], in_=ot[:, 0:h]
                    )
                    nc.vector.tensor_scalar_mul(
                        ot[:, h:C_OUT], po[:, h:C_OUT], OUT_SCALE
                    )
                    nc.scalar.dma_start(
                        out=out_d[mt * P : (mt + 1) * P, h:C_OUT],
                        in_=ot[:, h:C_OUT],
                    )

            with tc.tile_pool(name="out_psum2", bufs=6, space="PSUM") as outp2:
                for mt in range(NT_N):
                    if mt % 2 == 0:
                        out_group(accp, "acc", mt)
                    else:
                        out_group(outp2, "out2", mt)


_CACHED_NC = None


def _get_program():
    global _CACHED_NC
    if _CACHED_NC is None:
        _CACHED_NC = _build_program()
    return _CACHED_NC


def _pack_w(w, c_out, scale=1.0):
    # [c_out, C_IN] weight -> [128, NT_CIN*c_out] fp16, value at
    # [p, ct*c_out + m] = W[m, ct*128 + p]
    wt = (np.asarray(w, dtype=np.float32) * scale).astype(np.float16).T
    return np.ascontiguousarray(
        wt.reshape(NT_CIN, P, c_out).transpose(1, 0, 2).reshape(P, NT_CIN * c_out)
    )


def _pack_w_ck(w):
    # [C_KEY, C_IN] weight -> [128, NT_CK*NT_CIN*128] fp16, ck-tile-major
    wt = np.asarray(w, dtype=np.float32).astype(np.float16).T
    return np.ascontiguousarray(
        wt.reshape(NT_CIN, P, NT_CK, P)
        .transpose(1, 2, 0, 3)
        .reshape(P, NT_CK * NT_CIN * P)
    )


def _pack_x(xb):
    # [C_IN, N] -> [128, NJC*NT_CIN*JC] fp16, value at [p, (jc, ct, n)] =
    # x[ct*128 + p, jc*512 + n]
    return np.ascontiguousarray(
        xb.reshape(NT_CIN, P, NJC, JC)
        .transpose(1, 2, 0, 3)
        .reshape(P, NJC * NT_CIN * JC)
    )


def run(inputs, trace=False):
    nc = _get_program()
    x = np.asarray(inputs["x"], dtype=np.float32).astype(np.float16)
    wq_p = _pack_w_ck(inputs["Wq"])
    wk_p = _pack_w_ck(inputs["Wk"])
    wqk = np.ascontiguousarray(np.concatenate([wq_p, wk_p], axis=1))
    wv = _pack_w(inputs["Wv"], C_OUT, scale=1.0 / OUT_SCALE)
    in_maps = [{"x": _pack_x(x[b]), "wqk": wqk, "wv": wv} for b in range(B)]
    res = run_bass_kernel_spmd(nc, in_maps, core_ids=list(range(B)), trace=trace)
    out = np.stack(
        [np.asarray(res.results[b]["out"], dtype=np.float32) for b in range(B)]
    )
    return out, res


def kernel(x, Wq, Wk, Wv):
    out, _ = run({"x": x, "Wq": Wq, "Wk": Wk, "Wv": Wv}, trace=False)
    return out


# revision 37
# speedup vs baseline: 1.0089x; 1.0089x over previous
"""Trainium2 Bass kernel for an attention layer.

Computes, per batch element b (8 batches, one per NeuronCore):
    q = Wq @ x[b]            # [256, 2048]
    k = Wk @ x[b]            # [256, 2048]
    v = Wv @ x[b]            # [512, 2048]
    sim = k.T @ q            # [2048, 2048]
    attn = softmax(sim, -1)
    out[b] = (v @ attn).T    # [2048, 512]

Sharding: data-parallel over batch B=8 across the 8 cores; no collectives.

Per-core dataflow (fp16 matmuls, fp32 PSUM accumulation) with two headline
optimizations over the plain fp16 pipeline:

1. h-split emission order for DMA slack.  The first NSPLIT key-tiles'
   sim work is split by j-halves: proj jc0/jc1 -> sim h0 (cols 0..1023)
   for tiles 0..7 -> proj jc2/jc3 -> sim h1 + stats + vT for tiles 0..7
   -> tiles 8..15 whole.  The 32 h0 matmuls consume no fresh DMA input,
   giving the x jc1/jc2/jc3 and wv transfers a wide window; pieces are
   queued per-queue in consumption order (3 queues: sync/scalar HWDGE,
   gpsimd SWDGE), so the PE's input-DMA stalls drop from ~3.8us to
   ~1-2us of first-piece jitter.

2. fp8 DoubleRow for half the attention*V contraction.  A DoubleRow
   e4m3 matmul contracts 256 keys per 512-cycle instruction (2 fp8
   weights/cell; measured at bf16's per-instruction cost, i.e. true 2x
   MACs), so key-tiles stored fp8 cost half the out-phase matmul
   slots.  Accuracy only allows a partial conversion: e4m3's ~2.6% rms
   quantization noise on the V operand adds ~2.5e-2 L2 error if
   applied to the whole contraction (gate: 2e-2), so only the LAST
   2*N8PAIR key-tiles go fp8: rel err ~= sqrt(3.3e-3^2 +
   (2.5e-2)^2 * N8PAIR/8); N8PAIR=4 measures 1.863e-2 on HW and saves
   64 of 256 out-phase matmul slots.  The attention-weight side is
   nearly free: p8 = e4m3(exp_s * 128/emax) puts each row's max weight
   exactly on 128 (an e4m3 grid point, zero quantization error on the
   dominant weight), and v8 = vp * rden * emax/128 so the emax factor
   cancels exactly in the p8 @ v8 product.  v8 = 32 v / denrow stays
   within +-166 (TRN e4m3 maps >248 to Inf).  Wv is host-scaled by
   4096 so fp8 and bf16 tile contributions share one PSUM scale; the
   final copies multiply by 2^-12.

Engine balance (the PE must never wait):
  - All softmax denominators ride the ACT accumulator (both halves);
    the DVE never reduces exp_s for sums.
  - The fp8 tiles are the LAST 8: their conversion chains (per-half
    emax reduces + p8/v8 scalar-muls, ~5us of DVE per tile vs 2.6us of
    PE) drain into the out phase, whose first groups lead with the
    early-ready bf16 tiles; the final output copies run on ACT
    (activation Copy with immediate scale), which idles there, keeping
    the DVE free for the conversion backlog.
  - sim PSUM pool has 3 bufs; the vT accumulators share the 2-buf
    "acc" pool (8 banks total).  After the sim pool closes, the out
    phase runs 8 accumulation groups deep (2 on "acc" + 6 on the
    out_psum2 pool reusing the sim banks), enough runway to hide the
    fp8 conversion backlog and the sim->out release barrier (-4us).

Timing notes (8-core SPMD; the chip alternates between 2.4 GHz and a
2.0 GHz chip-wide power-throttle state, so a 512-col MM paces at
216-259 ns and identical builds measure 120-146us):
  - ~7us fixed NEFF/engine-init preamble precedes the first user op;
    3 wide + 7 short warmup matmuls on a zeroed tile bridge engine
    release to the first input pieces landing, and the HAM clock-ramp
    window (~3.4us) overlaps real projection work.
  - 448 real matmul slots (64 proj + 128 sim + 64 vT + 192 out) at
    216 ns = 96.8us of PE floor at 2.4 GHz.
  - The final output tile is halved into ACT+DVE scaled casts whose
    DMAs trigger on two different queues.
"""

import os

import numpy as np

import concourse.tile as tile
from concourse import bacc, mybir
from concourse.bass_utils import run_bass_kernel_spmd

B = 8
C_IN = 512
C_OUT = 512
C_KEY = 256
N = 2048
P = 128

F32 = mybir.dt.float32
F16 = mybir.dt.float16
BF16 = mybir.dt.bfloat16
E4 = mybir.dt.float8e4

DR = mybir.MatmulPerfMode.DoubleRow

NT_CIN = C_IN // P  # 4 tiles over input channels
NT_CK = C_KEY // P  # 2 tiles over key channels
NT_N = N // P  # 16 tiles over sequence positions
JC = 512  # matmul output chunk (one PSUM bank of fp32)
NJC = N // JC  # 4 chunks over the j axis
HC = 1024  # softmax processing chunk (half row block)
NHC = N // HC

EXP_SHIFT = -65.0  # global logit shift; row maxes are ~[38, 103] for this
# problem's N(0,1) inputs, and bf16/fp32 exponent range absorbs e^+-40

# Number of fp8 DoubleRow key-tile pairs (0..4).  Each pair halves 32 of the
# out-phase matmul slots.  Accuracy: ~sqrt(3.3e-3^2 + 6.5e-4 * n8).
N8PAIR = int(os.environ.get("KERNEL_N8PAIR", "4"))
NF8 = 2 * N8PAIR
NSPLIT = 8  # tiles 0..7 are h-split (their h0 sim runs before proj jc2/3)

OUT_SCALE = 1.0 / 4096.0  # undo the host-side Wv*4096 scale


def _build_program(n8pair=None):
    n8 = N8PAIR if n8pair is None else n8pair
    nc = bacc.Bacc("TRN2", target_bir_lowering=False, debug=False)

    # Host-packed inputs: every DRAM tensor is laid out so each SBUF
    # partition's data is one long contiguous run (DMA cost is dominated by
    # descriptor count; short runs halve the effective queue bandwidth).
    #   x:   [128, 4*4*512]  ([p, jc, ct, n] flattened)
    #   wqk: [128, 2*2*4*128]  ([p, w, ckt, ct, m] flattened)
    #   wv:  [128, 4*512]    ([p, ct, co] flattened), host-scaled by 4096
    x_d = nc.dram_tensor(
        "x", [P, NJC * NT_CIN * JC], F16, kind="ExternalInput"
    ).ap()
    wqk_d = nc.dram_tensor(
        "wqk", [P, 2 * NT_CIN * C_KEY], F16, kind="ExternalInput"
    ).ap()
    wv_d = nc.dram_tensor("wv", [P, NT_CIN * C_OUT], F16, kind="ExternalInput").ap()
    out_d = nc.dram_tensor("out", [N, C_OUT], BF16, kind="ExternalOutput").ap()

    with tile.TileContext(nc) as tc:
        _emit_kernel(tc, out_d, x_d, wqk_d, wv_d, n8)

    nc.compile()
    return nc


def _emit_kernel(tc, out_d, x_d, wqk_d, wv_d, n8):
    nc = tc.nc
    Exp = mybir.ActivationFunctionType.Exp
    AxisX = mybir.AxisListType.X
    Add = mybir.AluOpType.add
    Max = mybir.AluOpType.max
    nf8 = 2 * n8

    with (
        tc.tile_pool(name="persist", bufs=1) as persist,
        tc.tile_pool(name="stats", bufs=8) as stats,
        tc.tile_pool(name="ostage", bufs=6) as ostage,
    ):
        # ---- constants & warmup source (vector ops run before its DMAs) ----
        warm_src = persist.tile([P, JC], F16, tag="warm_src")
        nc.vector.memset(warm_src, 0.0)
        shift_bias = persist.tile([P, 1], F32, tag="shift")
        nc.vector.memset(shift_bias, EXP_SHIFT)

        # ---- input staging tiles ----
        x4 = persist.tile([P, NJC, NT_CIN, JC], F16, tag="x4")
        wqk_s = persist.tile([P, 2, NT_CK, NT_CIN, P], F16, tag="wqk")
        wv_s = persist.tile([P, NT_CIN, C_OUT], F16, tag="wv")

        def xp(ct, lo, hi):
            jc, r = divmod(lo, JC)
            assert hi - lo <= JC - r
            return x4[:, jc, ct, r : r + (hi - lo)]

        def x_piece(eng, jc, ct0, nt):
            return eng.dma_start(
                out=x4[:, jc, ct0 : ct0 + nt, :],
                in_=x_d[
                    :,
                    (jc * NT_CIN + ct0) * JC : (jc * NT_CIN + ct0 + nt) * JC,
                ].rearrange("p (t n) -> p t n", t=nt),
            )

        def wqk_piece(eng, w, ckt):
            WB = NT_CIN * P
            base = (w * NT_CK + ckt) * WB
            return eng.dma_start(
                out=wqk_s[:, w, ckt, :, :],
                in_=wqk_d[:, base : base + WB].rearrange("p (t m) -> p t m", t=NT_CIN),
            )

        # DMA schedule: per-queue FIFO order == consumption order (3 HW
        # queues: sync/scalar HWDGE + gpsimd SWDGE).
        # criticals: wqk00+x jc0 by ~9us, wqk01 ~10.5, wqk10 ~11.7,
        # wqk11 ~12.4, x jc1 ~13.5, x jc2 ~23, x jc3 ~26.5, wv ~30.
        x_piece(nc.sync, 0, 0, 1)
        wqk_piece(nc.scalar, 0, 0)
        x_piece(nc.gpsimd, 0, 1, 1)
        x_piece(nc.sync, 0, 2, 1)
        wqk_piece(nc.scalar, 0, 1)
        x_piece(nc.gpsimd, 0, 3, 1)
        wqk_piece(nc.sync, 1, 0)
        x_piece(nc.scalar, 1, 0, 2)
        wqk_piece(nc.gpsimd, 1, 1)
        x_piece(nc.sync, 1, 2, 2)
        x_piece(nc.sync, 2, 0, NT_CIN)
        x_piece(nc.gpsimd, 3, 0, NT_CIN)
        nc.scalar.dma_start(
            out=wv_s, in_=wv_d.rearrange("p (t m) -> p t m", t=NT_CIN)
        )
        # preload ACT's Exp table during the DMA wait (one-time ~1.3us load)
        act_warm = stats.tile([P, 1], F32, tag="actwarm")
        nc.scalar.activation(
            out=act_warm, in_=shift_bias, func=Exp, bias=shift_bias, scale=1.0
        )

        # ---- persistent compute tiles ----
        qs = [
            persist.tile([P, N], F16, tag=f"q{t}", name=f"q{t}") for t in range(NT_CK)
        ]
        ks = [
            persist.tile([P, N], F16, tag=f"k{t}", name=f"k{t}") for t in range(NT_CK)
        ]
        exp_s = [
            persist.tile([P, N], BF16, tag=f"e{it}", name=f"e{it}")
            for it in range(NT_N)
        ]
        vts = [
            persist.tile([P, C_OUT], BF16, tag=f"vt{it}", name=f"vt{it}")
            for it in range(NT_N)
        ]
        # fp8 DoubleRow pair tiles: plane = key-tile parity within the pair.
        # fp8 tiles are the LAST nf8 key-tiles: their conversion chains run
        # while earlier bf16 tiles already feed the out phase.
        f8_base = NT_N - nf8
        exp8 = [
            persist.tile([P, 2, N], E4, tag=f"e8{tp}", name=f"e8{tp}")
            for tp in range(n8)
        ]
        vt8 = [
            persist.tile([P, 2, C_OUT], E4, tag=f"v8{tp}", name=f"v8{tp}")
            for tp in range(n8)
        ]
        # per-tile softmax partial sums (ACT accumulates both halves)
        dpall = persist.tile([P, NT_N, 2], F32, tag="dpall")
        # per-half exp row-maxes for the fp8 tiles (the h0 half runs during
        # the tile's own h1 matmuls, shortening the tail conversion chain)
        em2 = persist.tile([P, max(nf8, 1), 2], F32, tag="em2")
        rs_all = persist.tile([P, max(nf8, 1)], F32, tag="rs_all")

        def sim_half(simp, it, h, name):
            sh = simp.tile([P, HC], F32, tag="sim", name=name)
            for jc in range(HC // JC):
                for ckt in range(NT_CK):
                    nc.tensor.matmul(
                        out=sh[:, jc * JC : (jc + 1) * JC],
                        lhsT=ks[ckt][:, it * P : (it + 1) * P],
                        rhs=qs[ckt][
                            :, (h * HC + jc * JC) : (h * HC + (jc + 1) * JC)
                        ],
                        start=(ckt == 0),
                        stop=(ckt == NT_CK - 1),
                    )
            return sh

        def vt_matmuls(accp, it):
            vp = accp.tile([P, C_OUT], F32, tag="acc", name=f"vp{it}")
            for ct in range(NT_CIN):
                nc.tensor.matmul(
                    out=vp,
                    lhsT=xp(ct, it * P, (it + 1) * P),
                    rhs=wv_s[:, ct, :],
                    start=(ct == 0),
                    stop=(ct == NT_CIN - 1),
                )
            return vp

        # stats+scale chain shared by all tiles; fp8 tiles additionally
        # derive the DoubleRow operands from exp_s/vp:
        #   emax = rowmax(exp_s)      (bf16 SBUF read, no PSUM retention)
        #   t0 = emax/128, rs = 1/t0  (p8 = exp_s*rs puts the row max at 128,
        #                              an exact e4m3 grid point)
        #   rden8 = rden*t0           (so rs cancels exactly in p8 @ v8)
        def tile_stats(accp, it):
            fp8 = it >= f8_base
            den = stats.tile([P, 1], F32, tag="den")
            rden = stats.tile([P, 1], F32, tag="rden")
            nc.vector.tensor_reduce(
                out=den, in_=dpall[:, it, :], axis=AxisX, op=Add
            )
            nc.vector.reciprocal(out=rden, in_=den)
            vp = vt_matmuls(accp, it)
            if fp8:
                fi = it - f8_base
                emax = stats.tile([P, 1], F32, tag="emax")
                nc.vector.tensor_reduce(
                    out=emax, in_=em2[:, fi, :], axis=AxisX, op=Max
                )
                t0 = stats.tile([P, 1], F32, tag="t0")
                nc.vector.tensor_scalar_mul(t0, emax, 1.0 / 128.0)
                rs = rs_all[:, fi : fi + 1]
                nc.vector.reciprocal(out=rs, in_=t0)
                rden8 = stats.tile([P, 1], F32, tag="rden8")
                nc.vector.tensor_scalar_mul(rden8, rden, t0)
                nc.vector.tensor_scalar_mul(vt8[fi // 2][:, fi % 2, :], vp, rden8)
                if fi >= 4:
                    nc.vector.tensor_scalar_mul(
                        exp8[fi // 2][:, fi % 2, :], exp_s[it], rs
                    )
            else:
                nc.vector.tensor_scalar_mul(vts[it], vp, rden)

        with tc.tile_pool(name="acc_psum", bufs=2, space="PSUM") as accp:
            # PE warmup bridging engine-release to the first input landing;
            # the HAM clock ramp overlaps the first real projections.
            warm_ps = accp.tile([P, JC], F32, tag="acc", name="warm_ps")
            for i in range(10):
                nc.tensor.matmul(
                    out=warm_ps[:, 0 : (JC if i < 3 else JC // 4)],
                    lhsT=warm_src[:, 0:P],
                    rhs=warm_src[:, 0 : (JC if i < 3 else JC // 4)],
                    start=True,
                    stop=True,
                )

            def proj_jc(jc):
                for w, dst in ((0, qs), (1, ks)):
                    for ckt in range(NT_CK):
                        ps = accp.tile([P, JC], F32, tag="acc", name=f"pj{jc}{w}{ckt}")
                        for ct in range(NT_CIN):
                            nc.tensor.matmul(
                                out=ps,
                                lhsT=wqk_s[:, w, ckt, ct, :],
                                rhs=x4[:, jc, ct, :],
                                start=(ct == 0),
                                stop=(ct == NT_CIN - 1),
                            )
                        nc.vector.tensor_copy(
                            out=dst[ckt][:, jc * JC : (jc + 1) * JC], in_=ps
                        )

            proj_jc(0)
            proj_jc(1)

            with tc.tile_pool(name="sim_psum", bufs=3, space="PSUM") as simp:
                # ---- h0 block: sim cols 0..1023 for the split tiles ----
                for it in range(NSPLIT):
                    sh = sim_half(simp, it, 0, f"s0_{it}")
                    nc.scalar.activation(
                        out=exp_s[it][:, 0:HC],
                        in_=sh,
                        func=Exp,
                        bias=shift_bias,
                        scale=1.0,
                        accum_out=dpall[:, it, 0:1],
                    )

                proj_jc(2)
                proj_jc(3)

                # ---- h1 block: sim cols 1024..2047 + stats + vT ----
                for it in range(NSPLIT):
                    sh = sim_half(simp, it, 1, f"s1_{it}")
                    nc.scalar.activation(
                        out=exp_s[it][:, HC:N],
                        in_=sh,
                        func=Exp,
                        bias=shift_bias,
                        scale=1.0,
                        accum_out=dpall[:, it, 1:2],
                    )
                    tile_stats(accp, it)

                # ---- tiles 8..15: whole-row processing; the fp8 tiles'
                # conversion chains drain into the out-phase runway ----
                for it in range(NSPLIT, NT_N):
                    for h in range(NHC):
                        sh = sim_half(simp, it, h, f"sf{it}_{h}")
                        nc.scalar.activation(
                            out=exp_s[it][:, h * HC : (h + 1) * HC],
                            in_=sh,
                            func=Exp,
                            bias=shift_bias,
                            scale=1.0,
                            accum_out=dpall[:, it, h : h + 1],
                        )
                        if it >= f8_base:
                            nc.vector.tensor_reduce(
                                out=em2[:, it - f8_base, h : h + 1],
                                in_=exp_s[it][:, h * HC : (h + 1) * HC],
                                axis=AxisX,
                                op=Max,
                            )
                    tile_stats(accp, it)

            # ---- out[m, co]: bf16 tiles via fp16 MMs, fp8 pairs via
            # DoubleRow (256-key contraction per instruction) ----
            def out_group(pool, tg, mt):
                po = pool.tile([P, C_OUT], F32, tag=tg, name=f"po{mt}")
                nmm = (NT_N - nf8) + n8
                mi = 0
                for it in range(f8_base):
                    nc.tensor.matmul(
                        out=po,
                        lhsT=exp_s[it][:, mt * P : (mt + 1) * P],
                        rhs=vts[it],
                        start=(mi == 0),
                        stop=(mi == nmm - 1),
                    )
                    mi += 1
                for tp in range(n8):
                    nc.tensor.matmul(
                        out=po,
                        lhsT=exp8[tp][:, :, mt * P : (mt + 1) * P],
                        rhs=vt8[tp],
                        start=(mi == 0),
                        stop=(mi == nmm - 1),
                        perf_mode=DR,
                    )
                    mi += 1
                ot = ostage.tile([P, C_OUT], BF16, tag="ostage", name=f"ot{mt}")
                Copy = mybir.ActivationFunctionType.Copy
                if mt < NT_N - 1:
                    # scaled final copy on ACT (idle in the out phase; the
                    # DVE is busy draining the fp8 conversion backlog)
                    deng = nc.sync if mt % 2 == 0 else nc.scalar
                    nc.scalar.activation(
                        out=ot, in_=po, func=Copy, bias=0.0, scale=OUT_SCALE
                    )
                    deng.dma_start(out=out_d[mt * P : (mt + 1) * P, :], in_=ot)
                else:
                    # final tile: halve it, casts split across ACT and DVE,
                    # DMAs on two different queues
                    h = C_OUT // 2
                    nc.scalar.activation(
                        out=ot[:, 0:h],
                        in_=po[:, 0:h],
                        func=Copy,
                        bias=0.0,
                        scale=OUT_SCALE,
                    )
                    nc.sync.dma_start(
                        out=out_d[mt * P : (mt + 1) * P, 0:h], in_=ot[:, 0:h]
                    )
                    nc.vector.tensor_scalar_mul(
                        ot[:, h:C_OUT], po[:, h:C_OUT], OUT_SCALE
                    )
                    nc.scalar.dma_start(
                        out=out_d[mt * P : (mt + 1) * P, h:C_OUT],
                        in_=ot[:, h:C_OUT],
                    )

            for fi in range(min(4, nf8)):
                nc.vector.tensor_scalar_mul(
                    exp8[fi // 2][:, fi % 2, :],
                    exp_s[f8_base + fi],
                    rs_all[:, fi : fi + 1],
                )
            with tc.tile_pool(name="out_psum2", bufs=6, space="PSUM") as outp2:
                for mt in range(NT_N):
                    if mt % 2 == 0:
                        out_group(accp, "acc", mt)
                    else:
                        out_group(outp2, "out2", mt)


_CACHED_NC = None


def _get_program():
    global _CACHED_NC
    if _CACHED_NC is None:
        _CACHED_NC = _build_program()
    return _CACHED_NC


def _pack_w(w, c_out, scale=1.0):
    # [c_out, C_IN] weight -> [128, NT_CIN*c_out] fp16, value at
    # [p, ct*c_out + m] = W[m, ct*128 + p]
    wt = (np.asarray(w, dtype=np.float32) * scale).astype(np.float16).T
    return np.ascontiguousarray(
        wt.reshape(NT_CIN, P, c_out).transpose(1, 0, 2).reshape(P, NT_CIN * c_out)
    )


def _pack_w_ck(w):
    # [C_KEY, C_IN] weight -> [128, NT_CK*NT_CIN*128] fp16, ck-tile-major
    wt = np.asarray(w, dtype=np.float32).astype(np.float16).T
    return np.ascontiguousarray(
        wt.reshape(NT_CIN, P, NT_CK, P)
        .transpose(1, 2, 0, 3)
        .reshape(P, NT_CK * NT_CIN * P)
    )


def _pack_x(xb):
    # [C_IN, N] -> [128, NJC*NT_CIN*JC] fp16, value at [p, (jc, ct, n)] =
    # x[ct*128 + p, jc*512 + n]
    return np.ascontiguousarray(
        xb.reshape(NT_CIN, P, NJC, JC)
        .transpose(1, 2, 0, 3)
        .reshape(P, NJC * NT_CIN * JC)
    )


def run(inputs, trace=False):
    nc = _get_program()
    x = np.asarray(inputs["x"], dtype=np.float32).astype(np.float16)
    wq_p = _pack_w_ck(inputs["Wq"])
    wk_p = _pack_w_ck(inputs["Wk"])
    wqk = np.ascontiguousarray(np.concatenate([wq_p, wk_p], axis=1))
    wv = _pack_w(inputs["Wv"], C_OUT, scale=1.0 / OUT_SCALE)
    in_maps = [{"x": _pack_x(x[b]), "wqk": wqk, "wv": wv} for b in range(B)]
    res = run_bass_kernel_spmd(nc, in_maps, core_ids=list(range(B)), trace=trace)
    out = np.stack(
        [np.asarray(res.results[b]["out"], dtype=np.float32) for b in range(B)]
    )
    return out, res


def kernel(x, Wq, Wk, Wv):
    out, _ = run({"x": x, "Wq": Wq, "Wk": Wk, "Wv": Wv}, trace=False)
    return out
